# revision 1
# baseline (speedup 1.0000x reference)
"""GroupQuantLinear on 8 Trainium2 NeuronCores.

y[b,s,o] = x[b,s,:] @ W[o,:] + bias[o], where W is dequantized on-device from
4-bit packed weights with per-(o, group) affine scale/bias (groups of 256 along
the 4096-wide input dim).

Sharding: tensor-parallel on out_features (8 shards of 2048 rows); x replicated.

Per-core kernel (Bass/Tile):
  Stage 1 (dequant): stream packed int32 words [o-tile 128, 1024 words],
    unpack 4 nibble planes with one fused DVE tensor_scalar (shift+and), then
    one fused DVE tensor_scalar (q * scale + wbias -> bf16) per (plane, group)
    with per-partition AP scalars.  Transpose the [o, in'] bf16 result to
    [in', o] via PE transposes, and store W^T into 4 DRAM quarter tensors.
  Stage 2 (matmul): composable_matmul_tile_kernel with kxm = x^T (f32 DMA +
    cast to bf16), kxn = streamed W^T quarters, fp32 PSUM accumulation, and the
    output bias folded into the PSUM->SBUF eviction (single DVE add).

Host marshalling is layout-only: x is transposed/permuted so the contraction
dim lands on SBUF partitions in the same nibble-plane-major order the on-chip
unpack produces (in' = plane*1024 + word, i.e. original index 4*word + plane).
"""

import numpy as np

B, S, IN, OUT, G = 2, 2048, 4096, 16384, 16
NCORES = 8
OSH = OUT // NCORES       # 2048 out rows per core
BS = B * S                # 4096
NW = IN // 4              # 1024 packed int32 words per out row
P = 128

_COMPILED = {}


def _build_nc():
    from contextlib import ExitStack

    import concourse.bass as bass
    import concourse.mybir as mybir
    import concourse.tile as tile
    from concourse import bacc
    from concourse.bass import ds, ts
    from concourse.masks import make_identity
    from concourse.kernels.tile_matmul import (
        ShapeInfo,
        cast_to_type,
        composable_matmul_tile_kernel,
        dma_from_dram_kxm,
        dma_to_dram_mxn,
    )

    f32 = mybir.dt.float32
    bf16 = mybir.dt.bfloat16
    i32 = mybir.dt.int32

    nc = bacc.Bacc(None, target_bir_lowering=False)

    xtp = nc.dram_tensor("xtp", [IN, BS], f32, kind="ExternalInput")
    wpk = nc.dram_tensor("wpk", [OSH, NW], i32, kind="ExternalInput")
    wsc = nc.dram_tensor("wsc", [OSH, G], f32, kind="ExternalInput")
    wbi = nc.dram_tensor("wbi", [OSH, G], f32, kind="ExternalInput")
    bias = nc.dram_tensor("bias", [1, OSH], f32, kind="ExternalInput")
    y = nc.dram_tensor("y", [BS, OSH], f32, kind="ExternalOutput")

    N_OT = OSH // P          # 16 o-tiles to dequantize
    N_KT = IN // 512         # 8 K tiles of 512
    GW = NW // G             # 64 words per group
    NWP = NW // P            # 8 in'-tiles per nibble plane
    NQ = OSH // 512          # 4 W^T quarter tensors

    with tile.TileContext(nc) as tc:
        with ExitStack() as ctx:
            const = ctx.enter_context(tc.tile_pool(name="const", bufs=1))
            dq = ctx.enter_context(tc.tile_pool(name="dq", bufs=2))
            dq_psum = ctx.enter_context(
                tc.tile_pool(name="dq_psum", bufs=2, space="PSUM")
            )
            dram = ctx.enter_context(tc.tile_pool(name="wt_dram", bufs=1, space="DRAM"))

            # ---- bias broadcast to [P, OSH] via K=1 fp32 matmuls ----
            bias_sb = const.tile([1, OSH], f32)
            nc.sync.dma_start(bias_sb[:], bias[:])
            ones_sb = const.tile([1, P], f32)
            nc.any.memset(ones_sb[:], 1.0)
            bias_bc = const.tile([P, OSH], f32)
            for j in range(OSH // 512):
                bps = dq_psum.tile([P, 512], f32, tag="biasps")
                nc.tensor.matmul(
                    bps[:], ones_sb[:], bias_sb[:, ts(j, 512)], start=True, stop=True
                )
                nc.any.tensor_copy(bias_bc[:, ts(j, 512)], bps[:])

            ident = const.tile([P, P], bf16)
            make_identity(nc, ident[:])

            # W^T quarters in DRAM: [IN, 512] each, rows in' plane-major order
            wt_q = [
                dram.tile([IN, 512], bf16, name=f"wt_q{i}") for i in range(NQ)
            ]

            # ---- Stage 1: dequant + transpose ----
            for ot in range(N_OT):
                osl = ts(ot, P)
                t_pk = dq.tile([P, NW], i32, tag="pk")
                nc.sync.dma_start(t_pk[:], wpk[osl, :])
                t_sc = dq.tile([P, G], f32, tag="sc")
                nc.sync.dma_start(t_sc[:], wsc[osl, :])
                t_bi = dq.tile([P, G], f32, tag="bi")
                nc.sync.dma_start(t_bi[:], wbi[osl, :])

                # wd[o, plane, w] bf16 == W'[o, in'] with in' = plane*NW + w
                wd = dq.tile([P, 4, NW], bf16, tag="wd")
                # unpack all 4 nibble planes (fused shift+and per plane)
                q4 = dq.tile([P, 4, NW], i32, tag="q4")
                for k in range(4):
                    nc.vector.tensor_scalar(
                        q4[:, k, :],
                        t_pk[:],
                        4 * k,
                        0xF,
                        mybir.AluOpType.logical_shift_right,
                        mybir.AluOpType.bitwise_and,
                    )
                # fused dequant, one DVE op per group across all 4 planes
                for g in range(G):
                    nc.vector.tensor_scalar(
                        wd[:, :, ts(g, GW)],
                        q4[:, :, ts(g, GW)],
                        t_sc[:, g : g + 1],
                        t_bi[:, g : g + 1],
                        mybir.AluOpType.mult,
                        mybir.AluOpType.add,
                    )

                # PE-transpose [o, in'] -> [in', o]; drain per K-tile of 512
                for kt in range(N_KT):
                    tps = dq_psum.tile([P, 4, P], bf16, tag="tps")
                    for s in range(4):
                        it = kt * 4 + s  # global in'-tile index
                        nc.tensor.transpose(
                            tps[:, s, :],
                            wd[:, it // NWP, ts(it % NWP, P)],
                            ident[:],
                        )
                    stg = dq.tile([P, 4, P], bf16, tag="stg")
                    nc.any.tensor_copy(stg[:], tps[:])
                    dst = wt_q[ot // 4].rearrange(
                        "(kt s p) c -> p kt s c", p=P, s=4
                    )[:, kt, :, ts(ot % 4, P)]
                    nc.sync.dma_start(dst, stg[:])

            # ---- Stage 2: matmul y = x @ W^T + bias ----
            kxm_pool = ctx.enter_context(tc.tile_pool(name="kxm", bufs=3))
            kxm_cast = ctx.enter_context(tc.tile_pool(name="kxmc", bufs=9))
            kxn_pool = ctx.enter_context(tc.tile_pool(name="kxn", bufs=9))

            kxm_producer, kxm_shape = dma_from_dram_kxm(kxm_pool, xtp[:])
            kxm_producer = cast_to_type(kxm_producer, kxm_cast, bf16)

            kxn_shape = ShapeInfo(pdims=((P, IN // P),), fdims=(OSH,))

            def kxn_producer(nc_, md):
                t = kxn_pool.tile([P, md.k_subtiles, md.n_tile], bf16, tag="kxn")
                src = wt_q[md.n_tile_idx].rearrange(
                    "(kt s p) c -> p kt s c", p=P, s=4
                )[:, md.k_tile_idx, :, :]
                nc_.sync.dma_start(t[:], src)
                return t

            def bias_evict(nc_, psum, sbuf, md):
                start = md.n_tile_idx * md.n_tile + md.n_subtile_idx * md.n_subtile
                nc_.vector.tensor_add(
                    sbuf, psum, bias_bc[:, ds(start, md.n_subtile)]
                )

            composable_matmul_tile_kernel(
                tc,
                kxm_shape=kxm_shape,
                kxn_shape=kxn_shape,
                output_type=f32,
                kxm_producer=kxm_producer,
                kxn_producer=kxn_producer,
                mxn_consumer=dma_to_dram_mxn(y[:]),
                mxn_subtile_reducer=bias_evict,
                psum_n_bufs=1,
                temps_n_bufs=2,
            )

    nc.compile()
    return nc


def _get_compiled():
    if "nc" not in _COMPILED:
        _COMPILED["nc"] = _build_nc()
    return _COMPILED["nc"]


def _marshal(input, w_packed, w_scale, w_bias, bias):
    x = np.ascontiguousarray(input, dtype=np.float32).reshape(BS, IN)
    # x^T with rows permuted to plane-major in' order: in' = k*NW + w <- 4w + k
    xt = x.T  # [IN, BS]
    xtp = np.ascontiguousarray(
        xt.reshape(NW, 4, BS).transpose(1, 0, 2).reshape(IN, BS)
    )
    in_maps = []
    for c in range(NCORES):
        osl = slice(c * OSH, (c + 1) * OSH)
        in_maps.append(
            {
                "xtp": xtp,
                "wpk": np.ascontiguousarray(w_packed[osl].reshape(OSH, NW)),
                "wsc": np.ascontiguousarray(w_scale[osl].reshape(OSH, G)),
                "wbi": np.ascontiguousarray(w_bias[osl].reshape(OSH, G)),
                "bias": np.ascontiguousarray(bias[osl].reshape(1, OSH)),
            }
        )
    return in_maps


def kernel(input, w_packed, w_scale, w_bias, bias, _trace=False, _trace_kwargs=None):
    from concourse.bass_utils import run_bass_kernel_spmd

    nc = _get_compiled()
    in_maps = _marshal(input, w_packed, w_scale, w_bias, bias)
    res = run_bass_kernel_spmd(
        nc,
        in_maps,
        core_ids=list(range(NCORES)),
        trace=_trace,
        **(_trace_kwargs or {}),
    )
    ys = [res.results[c]["y"] for c in range(NCORES)]
    out = np.concatenate(ys, axis=1).reshape(B, S, OUT).astype(np.float32)
    if _trace:
        return out, res
    return out



# revision 2
# speedup vs baseline: 1.1052x; 1.1052x over previous
"""GroupQuantLinear on 8 Trainium2 NeuronCores.

y[b,s,o] = x[b,s,:] @ W[o,:] + bias[o], where W is dequantized on-device from
4-bit packed weights with per-(o, group) affine scale/bias (groups of 256 along
the 4096-wide input dim).

Sharding: tensor-parallel on out_features (8 shards of 2048 rows); x replicated.

Per-core kernel (Bass/Tile), transpose-free design:
  The packed words are transposed on the HOST (layout-only) to [NW, OSH], so
  the on-chip nibble unpack lands directly in [in', out] orientation -- no PE
  transposes at all.  The per-(o, group) scale/bias are host-expanded to
  per-word rows [NW, OSH] (pure broadcast) so dequant is three DVE ops per
  tile: unpack (shift+and), t = q * scale, w = t + wbias, all at 16-bit DVE
  rate, written to a DRAM W^T tensor laid out so matmul-side DMAs are
  contiguous.  Dequant runs o-quarter-major so the first matmul column block
  is ready ~25 us in and the rest overlaps the matmul.

  Matmul: composable_matmul_tile_kernel, kxm = x^T bf16 (host-cast, row order
  in' = ksub*128 + p matching the unpack order), kxn = streamed W^T, fp32
  PSUM accumulation with 8-bank double buffering, output bias folded into the
  PSUM->SBUF eviction (single DVE add).

in' ordering: global k-subtile ksub = wt*4 + plane (wt = 128-word tile of the
packed words, plane = nibble index), so in' = wt*512 + plane*128 + p maps to
original input index 4*(wt*128 + p) + plane.
"""

import numpy as np

B, S, IN, OUT, G = 2, 2048, 4096, 16384, 16
NCORES = 8
OSH = OUT // NCORES       # 2048 out rows per core
BS = B * S                # 4096
NW = IN // 4              # 1024 packed int32 words per out row
P = 128
NKT = IN // 512           # 8 K tiles of 512 (4 k-subtiles each)
NQ = OSH // 512           # 4 o-quarters

_COMPILED = {}


def _build_nc():
    from contextlib import ExitStack

    import concourse.bass as bass
    import concourse.mybir as mybir
    import concourse.tile as tile
    from concourse import bacc
    from concourse.bass import ds, ts
    from concourse.kernels.tile_matmul import (
        ShapeInfo,
        composable_matmul_tile_kernel,
        dma_from_dram_kxm,
        dma_to_dram_mxn,
    )

    f32 = mybir.dt.float32
    bf16 = mybir.dt.bfloat16
    i32 = mybir.dt.int32

    nc = bacc.Bacc(None, target_bir_lowering=False)

    xtp = nc.dram_tensor("xtp", [IN, BS], bf16, kind="ExternalInput")
    wpkT = nc.dram_tensor("wpkT", [NW, OSH], i32, kind="ExternalInput")
    scx = nc.dram_tensor("scx", [NW, OSH], bf16, kind="ExternalInput")
    wbx = nc.dram_tensor("wbx", [NW, OSH], bf16, kind="ExternalInput")
    bias = nc.dram_tensor("bias", [1, OSH], f32, kind="ExternalInput")
    y = nc.dram_tensor("y", [BS, OSH], f32, kind="ExternalOutput")

    with tile.TileContext(nc) as tc:
        with ExitStack() as ctx:
            const = ctx.enter_context(tc.tile_pool(name="const", bufs=1))
            dq = ctx.enter_context(tc.tile_pool(name="dq", bufs=3))
            dram = ctx.enter_context(tc.tile_pool(name="wt_dram", bufs=1, space="DRAM"))

            # ---- bias broadcast to [P, OSH] via K=1 fp32 matmuls ----
            bias_sb = const.tile([1, OSH], f32)
            nc.sync.dma_start(bias_sb[:], bias[:])
            ones_sb = const.tile([1, P], f32)
            nc.any.memset(ones_sb[:], 1.0)
            bias_bc = const.tile([P, OSH], f32)
            with tc.tile_pool(name="bias_psum", bufs=1, space="PSUM") as bias_psum:
                for j in range(OSH // 512):
                    bps = bias_psum.tile([P, 512], f32, tag="biasps")
                    nc.tensor.matmul(
                        bps[:], ones_sb[:], bias_sb[:, ts(j, 512)], start=True, stop=True
                    )
                    nc.any.tensor_copy(bias_bc[:, ts(j, 512)], bps[:])

            # W^T in DRAM, matmul-tile-friendly layout: [p, kt, ksub, o]
            wt_all = dram.tile([P, NKT, 4, OSH], bf16, name="wt_all")

            # ---- Stage 1: dequant (o-quarter-major so matmul starts early) ----
            for j in range(NQ):
                osl = ts(j, 512)
                for wt in range(NKT):
                    rsl = ts(wt, P)
                    t_pk = dq.tile([P, 512], i32, tag="pk")
                    nc.sync.dma_start(t_pk[:], wpkT[rsl, osl])
                    t_sc = dq.tile([P, 512], bf16, tag="sc")
                    nc.sync.dma_start(t_sc[:], scx[rsl, osl])
                    t_wb = dq.tile([P, 512], bf16, tag="wb")
                    nc.sync.dma_start(t_wb[:], wbx[rsl, osl])

                    wd = dq.tile([P, 4, 512], bf16, tag="wd")
                    for plane in range(4):
                        q = dq.tile([P, 512], i32, tag="q")
                        nc.vector.tensor_scalar(
                            q[:],
                            t_pk[:],
                            4 * plane,
                            0xF,
                            mybir.AluOpType.logical_shift_right,
                            mybir.AluOpType.bitwise_and,
                        )
                        qf = dq.tile([P, 512], bf16, tag="qf")
                        nc.vector.tensor_copy(qf[:], q[:])
                        t = dq.tile([P, 512], bf16, tag="t")
                        nc.vector.tensor_tensor(
                            t[:], qf[:], t_sc[:], mybir.AluOpType.mult
                        )
                        nc.vector.tensor_tensor(
                            wd[:, plane, :], t[:], t_wb[:], mybir.AluOpType.add
                        )
                    nc.sync.dma_start(wt_all[:, wt, :, osl], wd[:])

            # ---- Stage 2: matmul y = x @ W^T + bias ----
            kxm_pool = ctx.enter_context(tc.tile_pool(name="kxm", bufs=10))
            kxn_pool = ctx.enter_context(tc.tile_pool(name="kxn", bufs=9))

            kxm_producer, kxm_shape = dma_from_dram_kxm(kxm_pool, xtp[:])
            kxn_shape = ShapeInfo(pdims=((P, IN // P),), fdims=(OSH,))

            def kxn_producer(nc_, md):
                t = kxn_pool.tile([P, md.k_subtiles, md.n_tile], bf16, tag="kxn")
                nc_.sync.dma_start(
                    t[:],
                    wt_all[:, md.k_tile_idx, :, ds(md.n_tile_idx * md.n_tile, md.n_tile)],
                )
                return t

            def bias_evict(nc_, psum, sbuf, md):
                start = md.n_tile_idx * md.n_tile + md.n_subtile_idx * md.n_subtile
                nc_.vector.tensor_add(
                    sbuf, psum, bias_bc[:, ds(start, md.n_subtile)]
                )

            composable_matmul_tile_kernel(
                tc,
                kxm_shape=kxm_shape,
                kxn_shape=kxn_shape,
                output_type=f32,
                kxm_producer=kxm_producer,
                kxn_producer=kxn_producer,
                mxn_consumer=dma_to_dram_mxn(y[:]),
                mxn_subtile_reducer=bias_evict,
                psum_n_bufs=2,
                temps_n_bufs=2,
            )

    nc.compile()
    return nc


def _get_compiled():
    if "nc" not in _COMPILED:
        _COMPILED["nc"] = _build_nc()
    return _COMPILED["nc"]


def _marshal(input, w_packed, w_scale, w_bias, bias):
    import ml_dtypes

    bf16 = ml_dtypes.bfloat16
    x = np.ascontiguousarray(input, dtype=np.float32).reshape(BS, IN)
    # x^T rows permuted so in' = (wt*4+plane)*128 + p <- original 4*(wt*128+p)+plane
    xt = x.T  # [IN, BS], row index = original in = 4*w + plane, w = wt*128 + p
    xtp = np.ascontiguousarray(
        xt.reshape(NKT, P, 4, BS).transpose(0, 2, 1, 3).reshape(IN, BS).astype(bf16)
    )
    in_maps = []
    for c in range(NCORES):
        osl = slice(c * OSH, (c + 1) * OSH)
        wp = w_packed[osl].reshape(OSH, NW)
        sc = w_scale[osl].reshape(OSH, G).astype(bf16)
        wb = w_bias[osl].reshape(OSH, G).astype(bf16)
        in_maps.append(
            {
                "xtp": xtp,
                "wpkT": np.ascontiguousarray(wp.T),
                "scx": np.ascontiguousarray(np.repeat(sc.T, NW // G, axis=0)),
                "wbx": np.ascontiguousarray(np.repeat(wb.T, NW // G, axis=0)),
                "bias": np.ascontiguousarray(bias[osl].reshape(1, OSH), dtype=np.float32),
            }
        )
    return in_maps


def kernel(input, w_packed, w_scale, w_bias, bias, _trace=False, _trace_kwargs=None):
    from concourse.bass_utils import run_bass_kernel_spmd

    nc = _get_compiled()
    in_maps = _marshal(input, w_packed, w_scale, w_bias, bias)
    res = run_bass_kernel_spmd(
        nc,
        in_maps,
        core_ids=list(range(NCORES)),
        trace=_trace,
        **(_trace_kwargs or {}),
    )
    ys = [res.results[c]["y"] for c in range(NCORES)]
    out = np.concatenate(ys, axis=1).reshape(B, S, OUT).astype(np.float32)
    if _trace:
        return out, res
    return out


# revision 4
# speedup vs baseline: 1.1111x; 1.0053x over previous
"""GroupQuantLinear on 8 Trainium2 NeuronCores.

y[b,s,o] = x[b,s,:] @ W[o,:] + bias[o], where W is dequantized on-device from
4-bit packed weights with per-(o, group) affine scale/bias (groups of 256 along
the 4096-wide input dim).

Sharding: tensor-parallel on out_features (8 shards of 2048 rows); x replicated.

Per-core kernel (Bass/Tile), transpose-free, W-stationary design:
  The packed words are transposed on the HOST (layout-only) to [NW, OSH], so
  the on-chip nibble unpack lands directly in [in', out] orientation -- no PE
  transposes.  Per-(o, group) scale/bias are host-expanded to per-word rows
  (pure broadcast) so dequant is three 16-bit-rate DVE ops per tile: unpack
  (fused shift+and, bf16 out), t = q * scale, w = t + wbias, written to a
  DRAM W^T tensor laid out so matmul-side DMAs are contiguous.

  Matmul computes y^T = W @ x^T: kxm = W^T (cached per o-quarter m-tile),
  kxn = x^T bf16 streamed (host-cast, row order in' = ksub*128 + p matching
  the unpack order).  m-tiles align 1:1 with dequant o-quarters, so the first
  quarter unlocks a full 218 us row of PE work and the rest of dequant hides
  under it.  PSUM partitions carry o, so the output bias is a per-partition
  tensor_scalar add at eviction.  Host un-transposes y^T.

in' ordering: global k-subtile ksub = wt*4 + plane (wt = 128-word tile of the
packed words, plane = nibble index), so in' = wt*512 + plane*128 + p maps to
original input index 4*(wt*128 + p) + plane.
"""

import numpy as np

B, S, IN, OUT, G = 2, 2048, 4096, 16384, 16
NCORES = 8
OSH = OUT // NCORES       # 2048 out rows per core
BS = B * S                # 4096
NW = IN // 4              # 1024 packed int32 words per out row
P = 128
NKT = IN // 512           # 8 K tiles of 512 (4 k-subtiles each)
NQ = OSH // 512           # 4 o-quarters = matmul m-tiles

_COMPILED = {}


def _build_nc():
    from contextlib import ExitStack

    import concourse.bass as bass
    import concourse.mybir as mybir
    import concourse.tile as tile
    from concourse import bacc
    from concourse.bass import ds, ts
    from concourse.kernels.tile_matmul import (
        ShapeInfo,
        composable_matmul_tile_kernel,
        dma_from_dram_kxn,
        dma_to_dram_mxn,
    )

    f32 = mybir.dt.float32
    bf16 = mybir.dt.bfloat16
    i32 = mybir.dt.int32

    nc = bacc.Bacc(None, target_bir_lowering=False)

    xtp = nc.dram_tensor("xtp", [IN, BS], bf16, kind="ExternalInput")
    wpkT = nc.dram_tensor("wpkT", [NW, OSH], i32, kind="ExternalInput")
    scx = nc.dram_tensor("scx", [NW, OSH], bf16, kind="ExternalInput")
    wbx = nc.dram_tensor("wbx", [NW, OSH], bf16, kind="ExternalInput")
    bias = nc.dram_tensor("bias", [P, OSH // P], f32, kind="ExternalInput")
    yT = nc.dram_tensor("yT", [OSH, BS], f32, kind="ExternalOutput")

    with tile.TileContext(nc) as tc:
        with ExitStack() as ctx:
            const = ctx.enter_context(tc.tile_pool(name="const", bufs=1))
            dq = ctx.enter_context(tc.tile_pool(name="dq", bufs=3))
            dram = ctx.enter_context(tc.tile_pool(name="wt_dram", bufs=1, space="DRAM"))

            # output bias, per-partition: bias_sb[p, j] = bias[j*128 + p]
            bias_sb = const.tile([P, OSH // P], f32)
            nc.sync.dma_start(bias_sb[:], bias[:])

            # W^T in DRAM, matmul-tile-friendly layout: [p, kt, ksub, o]
            wt_all = dram.tile([P, NKT, 4, OSH], bf16, name="wt_all")

            # ---- Stage 1: dequant (o-quarter-major = matmul m-tile order) ----
            for j in range(NQ):
                osl = ts(j, 512)
                for wt in range(NKT):
                    rsl = ts(wt, P)
                    t_pk = dq.tile([P, 512], i32, tag="pk")
                    nc.sync.dma_start(t_pk[:], wpkT[rsl, osl])
                    t_sc = dq.tile([P, 512], bf16, tag="sc")
                    nc.sync.dma_start(t_sc[:], scx[rsl, osl])
                    t_wb = dq.tile([P, 512], bf16, tag="wb")
                    nc.sync.dma_start(t_wb[:], wbx[rsl, osl])

                    wd = dq.tile([P, 4, 512], bf16, tag="wd")
                    for plane in range(4):
                        q = dq.tile([P, 512], i32, tag="q")
                        nc.vector.tensor_scalar(
                            q[:],
                            t_pk[:],
                            4 * plane,
                            0xF,
                            mybir.AluOpType.logical_shift_right,
                            mybir.AluOpType.bitwise_and,
                        )
                        qf = dq.tile([P, 512], bf16, tag="qf")
                        nc.any.tensor_copy(qf[:], q[:])
                        t = dq.tile([P, 512], bf16, tag="t")
                        nc.vector.tensor_tensor(
                            t[:], qf[:], t_sc[:], mybir.AluOpType.mult
                        )
                        nc.vector.tensor_tensor(
                            wd[:, plane, :], t[:], t_wb[:], mybir.AluOpType.add
                        )
                    nc.sync.dma_start(wt_all[:, wt, :, osl], wd[:])

            # ---- Stage 2: matmul y^T = W @ x^T (+bias at eviction) ----
            kxm_pool = ctx.enter_context(tc.tile_pool(name="kxm", bufs=10))
            kxn_pool = ctx.enter_context(tc.tile_pool(name="kxn", bufs=9))

            kxm_shape = ShapeInfo(pdims=((P, IN // P),), fdims=(OSH,))

            def kxm_producer(nc_, md):
                t = kxm_pool.tile([P, md.k_subtiles, md.m_tile], bf16, tag="kxm")
                nc_.sync.dma_start(
                    t[:],
                    wt_all[:, md.k_tile_idx, :, ds(md.m_tile_idx * md.m_tile, md.m_tile)],
                )
                return t

            kxn_producer, kxn_shape = dma_from_dram_kxn(kxn_pool, xtp[:])

            def bias_evict(nc_, psum, sbuf, md):
                ob = md.m_tile_idx * 4 + md.m_subtile_idx
                nc_.vector.tensor_scalar(
                    sbuf, psum, bias_sb[:, ob : ob + 1], None, mybir.AluOpType.add
                )

            composable_matmul_tile_kernel(
                tc,
                kxm_shape=kxm_shape,
                kxn_shape=kxn_shape,
                output_type=f32,
                kxm_producer=kxm_producer,
                kxn_producer=kxn_producer,
                mxn_consumer=dma_to_dram_mxn(yT[:]),
                mxn_subtile_reducer=bias_evict,
                psum_n_bufs=2,
                temps_n_bufs=2,
            )

    nc.compile()
    return nc


def _get_compiled():
    if "nc" not in _COMPILED:
        _COMPILED["nc"] = _build_nc()
    return _COMPILED["nc"]


def _marshal(input, w_packed, w_scale, w_bias, bias):
    import ml_dtypes

    bf16 = ml_dtypes.bfloat16
    x = np.ascontiguousarray(input, dtype=np.float32).reshape(BS, IN)
    # x^T rows permuted so in' = (wt*4+plane)*128 + p <- original 4*(wt*128+p)+plane
    xt = x.T  # [IN, BS], row index = original in = 4*w + plane, w = wt*128 + p
    xtp = np.ascontiguousarray(
        xt.reshape(NKT, P, 4, BS).transpose(0, 2, 1, 3).reshape(IN, BS).astype(bf16)
    )
    in_maps = []
    for c in range(NCORES):
        osl = slice(c * OSH, (c + 1) * OSH)
        wp = w_packed[osl].reshape(OSH, NW)
        sc = w_scale[osl].reshape(OSH, G).astype(bf16)
        wb = w_bias[osl].reshape(OSH, G).astype(bf16)
        in_maps.append(
            {
                "xtp": xtp,
                "wpkT": np.ascontiguousarray(wp.T),
                "scx": np.ascontiguousarray(np.repeat(sc.T, NW // G, axis=0)),
                "wbx": np.ascontiguousarray(np.repeat(wb.T, NW // G, axis=0)),
                "bias": np.ascontiguousarray(
                    bias[osl].reshape(OSH // P, P).T, dtype=np.float32
                ),
            }
        )
    return in_maps


def kernel(input, w_packed, w_scale, w_bias, bias, _trace=False, _trace_kwargs=None):
    from concourse.bass_utils import run_bass_kernel_spmd

    nc = _get_compiled()
    in_maps = _marshal(input, w_packed, w_scale, w_bias, bias)
    res = run_bass_kernel_spmd(
        nc,
        in_maps,
        core_ids=list(range(NCORES)),
        trace=_trace,
        **(_trace_kwargs or {}),
    )
    out = np.empty((BS, OUT), dtype=np.float32)
    for c in range(NCORES):
        out[:, c * OSH : (c + 1) * OSH] = res.results[c]["yT"].T
    out = out.reshape(B, S, OUT)
    if _trace:
        return out, res
    return out


# revision 7
# speedup vs baseline: 1.1508x; 1.0357x over previous
"""GroupQuantLinear on 8 Trainium2 NeuronCores.

y[b,s,o] = x[b,s,:] @ W[o,:] + bias[o], where W is dequantized on-device from
4-bit packed weights with per-(o, group) affine scale/bias (groups of 256 along
the 4096-wide input dim).

Sharding: tensor-parallel on out_features (8 shards of 2048 rows); x replicated.

Per-core kernel (Bass/Tile), transpose-free, W-stationary design:
  The packed words are transposed on the HOST (layout-only) to [NW, OSH], so
  the on-chip nibble unpack lands directly in [in', out] orientation -- no PE
  transposes.  Per-(o, group) scale/bias are host-expanded to per-word rows
  (pure broadcast) so dequant is three 16-bit-rate DVE ops per tile: unpack
  (fused shift+and, bf16 out), t = q * scale, w = t + wbias, written to a
  DRAM W^T tensor laid out so matmul-side DMAs are contiguous.

  Matmul computes y^T = W @ x^T: kxm = W^T (cached per o-quarter m-tile),
  kxn = x^T bf16 streamed (host-cast, row order in' = ksub*128 + p matching
  the unpack order).  m-tiles align 1:1 with dequant o-quarters, so the first
  quarter unlocks a full 218 us row of PE work and the rest of dequant hides
  under it.  PSUM partitions carry o, so the output bias is a per-partition
  tensor_scalar add at eviction.  Host un-transposes y^T.

in' ordering: global k-subtile ksub = wt*4 + plane (wt = 128-word tile of the
packed words, plane = nibble index), so in' = wt*512 + plane*128 + p maps to
original input index 4*(wt*128 + p) + plane.
"""

import numpy as np

B, S, IN, OUT, G = 2, 2048, 4096, 16384, 16
NCORES = 8
OSH = OUT // NCORES       # 2048 out rows per core
BS = B * S                # 4096
NW = IN // 4              # 1024 packed int32 words per out row
P = 128
NKT = IN // 512           # 8 K tiles of 512 (4 k-subtiles each)
NQ = OSH // 512           # 4 o-quarters = matmul m-tiles

_COMPILED = {}


def _build_nc():
    from contextlib import ExitStack

    import concourse.bass as bass
    import concourse.mybir as mybir
    import concourse.tile as tile
    from concourse import bacc
    from concourse.bass import ds, ts
    from concourse.kernels.tile_matmul import (
        ShapeInfo,
        composable_matmul_tile_kernel,
        dma_from_dram_kxn,
        dma_to_dram_mxn,
    )

    f32 = mybir.dt.float32
    bf16 = mybir.dt.bfloat16
    i32 = mybir.dt.int32

    nc = bacc.Bacc(None, target_bir_lowering=False)

    xtp = nc.dram_tensor("xtp", [IN, BS], bf16, kind="ExternalInput")
    wpkT = nc.dram_tensor("wpkT", [NW, OSH], i32, kind="ExternalInput")
    scx = nc.dram_tensor("scx", [NW, OSH], bf16, kind="ExternalInput")
    wbx = nc.dram_tensor("wbx", [NW, OSH], bf16, kind="ExternalInput")
    bias = nc.dram_tensor("bias", [P, OSH // P], f32, kind="ExternalInput")
    yT = nc.dram_tensor("yT", [OSH, BS], f32, kind="ExternalOutput")

    with tile.TileContext(nc) as tc:
        with ExitStack() as ctx:
            const = ctx.enter_context(tc.tile_pool(name="const", bufs=1))
            dq = ctx.enter_context(tc.tile_pool(name="dq", bufs=3))

            # output bias, per-partition: bias_sb[p, j] = bias[j*128 + p]
            bias_sb = const.tile([P, OSH // P], f32)
            nc.sync.dma_start(bias_sb[:], bias[:])

            # W^T fully resident in SBUF (128 KB/partition): [p, kt, ksub, o]
            wt_res = const.tile([P, NKT, 4, OSH], bf16)

            # ---- Stage 1: dequant (o-quarter-major = matmul m-tile order) ----
            for j in range(NQ):
                osl = ts(j, 512)
                for wt in range(NKT):
                    rsl = ts(wt, P)
                    t_pk = dq.tile([P, 512], i32, tag="pk")
                    nc.sync.dma_start(t_pk[:], wpkT[rsl, osl])
                    t_sc = dq.tile([P, 512], bf16, tag="sc")
                    nc.sync.dma_start(t_sc[:], scx[rsl, osl])
                    t_wb = dq.tile([P, 512], bf16, tag="wb")
                    nc.sync.dma_start(t_wb[:], wbx[rsl, osl])

                    for plane in range(4):
                        q = dq.tile([P, 512], i32, tag="q")
                        nc.vector.tensor_scalar(
                            q[:],
                            t_pk[:],
                            4 * plane,
                            0xF,
                            mybir.AluOpType.logical_shift_right,
                            mybir.AluOpType.bitwise_and,
                        )
                        qf = dq.tile([P, 512], bf16, tag="qf")
                        nc.any.tensor_copy(qf[:], q[:])
                        t = dq.tile([P, 512], bf16, tag="t")
                        nc.vector.tensor_tensor(
                            t[:], qf[:], t_sc[:], mybir.AluOpType.mult
                        )
                        nc.vector.tensor_tensor(
                            wt_res[:, wt, plane, osl], t[:], t_wb[:], mybir.AluOpType.add
                        )

            # ---- Stage 2: matmul y^T = W @ x^T (+bias at eviction) ----
            kxn_pool = ctx.enter_context(tc.tile_pool(name="kxn", bufs=9))

            kxm_shape = ShapeInfo(pdims=((P, IN // P),), fdims=(OSH,))

            def kxm_producer(nc_, md):
                return wt_res[
                    :, md.k_tile_idx, :, ds(md.m_tile_idx * md.m_tile, md.m_tile)
                ]

            kxn_producer, kxn_shape = dma_from_dram_kxn(kxn_pool, xtp[:])

            def bias_evict(nc_, psum, sbuf, md):
                ob = md.m_tile_idx * 4 + md.m_subtile_idx
                nc_.vector.tensor_scalar(
                    sbuf, psum, bias_sb[:, ob : ob + 1], None, mybir.AluOpType.add
                )

            composable_matmul_tile_kernel(
                tc,
                kxm_shape=kxm_shape,
                kxn_shape=kxn_shape,
                output_type=f32,
                kxm_producer=kxm_producer,
                kxn_producer=kxn_producer,
                mxn_consumer=dma_to_dram_mxn(yT[:]),
                mxn_subtile_reducer=bias_evict,
                psum_n_bufs=2,
                temps_n_bufs=2,
            )

    nc.compile()
    return nc


def _get_compiled():
    if "nc" not in _COMPILED:
        _COMPILED["nc"] = _build_nc()
    return _COMPILED["nc"]


def _marshal(input, w_packed, w_scale, w_bias, bias):
    import ml_dtypes

    bf16 = ml_dtypes.bfloat16
    x = np.ascontiguousarray(input, dtype=np.float32).reshape(BS, IN)
    # x^T rows permuted so in' = (wt*4+plane)*128 + p <- original 4*(wt*128+p)+plane
    xt = x.T  # [IN, BS], row index = original in = 4*w + plane, w = wt*128 + p
    xtp = np.ascontiguousarray(
        xt.reshape(NKT, P, 4, BS).transpose(0, 2, 1, 3).reshape(IN, BS).astype(bf16)
    )
    in_maps = []
    for c in range(NCORES):
        osl = slice(c * OSH, (c + 1) * OSH)
        wp = w_packed[osl].reshape(OSH, NW)
        sc = w_scale[osl].reshape(OSH, G).astype(bf16)
        wb = w_bias[osl].reshape(OSH, G).astype(bf16)
        in_maps.append(
            {
                "xtp": xtp,
                "wpkT": np.ascontiguousarray(wp.T),
                "scx": np.ascontiguousarray(np.repeat(sc.T, NW // G, axis=0)),
                "wbx": np.ascontiguousarray(np.repeat(wb.T, NW // G, axis=0)),
                "bias": np.ascontiguousarray(
                    bias[osl].reshape(OSH // P, P).T, dtype=np.float32
                ),
            }
        )
    return in_maps


def kernel(input, w_packed, w_scale, w_bias, bias, _trace=False, _trace_kwargs=None):
    from concourse.bass_utils import run_bass_kernel_spmd

    nc = _get_compiled()
    in_maps = _marshal(input, w_packed, w_scale, w_bias, bias)
    res = run_bass_kernel_spmd(
        nc,
        in_maps,
        core_ids=list(range(NCORES)),
        trace=_trace,
        **(_trace_kwargs or {}),
    )
    out = np.empty((BS, OUT), dtype=np.float32)
    for c in range(NCORES):
        out[:, c * OSH : (c + 1) * OSH] = res.results[c]["yT"].T
    out = out.reshape(B, S, OUT)
    if _trace:
        return out, res
    return out


# revision 9
# speedup vs baseline: 1.2300x; 1.0688x over previous
"""GroupQuantLinear on 8 Trainium2 NeuronCores.

y[b,s,o] = x[b,s,:] @ W[o,:] + bias[o], where W is dequantized on-device from
4-bit packed weights with per-(o, group) affine scale/bias (groups of 256 along
the 4096-wide input dim).

Sharding: tensor-parallel on out_features (8 shards of 2048 rows); x replicated.

Per-core kernel (Bass/Tile), transpose-free, W-stationary design:
  The packed words are transposed on the HOST (layout-only) to [NW, OSH], so
  the on-chip nibble unpack lands directly in [in', out] orientation -- no PE
  transposes.  Per-(o, group) scale/bias are host-expanded to per-word rows
  (pure broadcast) so dequant is three 16-bit-rate DVE ops per tile: unpack
  (fused shift+and, bf16 out), t = q * scale, w = t + wbias, written to a
  DRAM W^T tensor laid out so matmul-side DMAs are contiguous.

  Matmul computes y^T = W @ x^T: kxm = W^T (cached per o-quarter m-tile),
  kxn = x^T bf16 streamed (host-cast, row order in' = ksub*128 + p matching
  the unpack order).  m-tiles align 1:1 with dequant o-quarters, so the first
  quarter unlocks a full 218 us row of PE work and the rest of dequant hides
  under it.  PSUM partitions carry o, so the output bias is a per-partition
  tensor_scalar add at eviction.  Host un-transposes y^T.

in' ordering: global k-subtile ksub = wt*4 + plane (wt = 128-word tile of the
packed words, plane = nibble index), so in' = wt*512 + plane*128 + p maps to
original input index 4*(wt*128 + p) + plane.
"""

import numpy as np

B, S, IN, OUT, G = 2, 2048, 4096, 16384, 16
NCORES = 8
OSH = OUT // NCORES       # 2048 out rows per core
BS = B * S                # 4096
NW = IN // 4              # 1024 packed int32 words per out row
P = 128
NKT = IN // 512           # 8 K tiles of 512 (4 k-subtiles each)
NQ = OSH // 512           # 4 o-quarters = matmul m-tiles

_COMPILED = {}


def _build_nc():
    from contextlib import ExitStack

    import concourse.bass as bass
    import concourse.mybir as mybir
    import concourse.tile as tile
    from concourse import bacc
    from concourse.bass import ds, ts
    from concourse.kernels.tile_matmul import (
        ShapeInfo,
        composable_matmul_tile_kernel,
        dma_from_dram_kxn,
        dma_to_dram_mxn,
    )

    f32 = mybir.dt.float32
    bf16 = mybir.dt.bfloat16
    i32 = mybir.dt.int32

    nc = bacc.Bacc(None, target_bir_lowering=False)

    xtp = nc.dram_tensor("xtp", [IN, BS], bf16, kind="ExternalInput")
    wpkT = nc.dram_tensor("wpkT", [NW, OSH], i32, kind="ExternalInput")
    scx = nc.dram_tensor("scx", [NW, OSH], bf16, kind="ExternalInput")
    wbx = nc.dram_tensor("wbx", [NW, OSH], bf16, kind="ExternalInput")
    bias = nc.dram_tensor("bias", [P, OSH // P], f32, kind="ExternalInput")
    yT = nc.dram_tensor("yT", [OSH, BS], f32, kind="ExternalOutput")

    with tile.TileContext(nc) as tc:
        with ExitStack() as ctx:
            const = ctx.enter_context(tc.tile_pool(name="const", bufs=1))
            dq = ctx.enter_context(tc.tile_pool(name="dq", bufs=3))

            # output bias, per-partition: bias_sb[p, j] = bias[j*128 + p]
            bias_sb = const.tile([P, OSH // P], f32)
            nc.sync.dma_start(bias_sb[:], bias[:])

            # W^T fully resident in SBUF (128 KB/partition): [p, kt, ksub, o]
            wt_res = const.tile([P, NKT, 4, OSH], bf16)

            # ---- Stage 1: dequant (o-quarter-major = matmul m-tile order) ----
            for j in range(NQ):
                osl = ts(j, 512)
                for wt in range(NKT):
                    rsl = ts(wt, P)
                    t_pk = dq.tile([P, 512], i32, tag="pk")
                    nc.sync.dma_start(t_pk[:], wpkT[rsl, osl])
                    t_sc = dq.tile([P, 512], bf16, tag="sc")
                    nc.sync.dma_start(t_sc[:], scx[rsl, osl])
                    t_wb = dq.tile([P, 512], bf16, tag="wb")
                    nc.sync.dma_start(t_wb[:], wbx[rsl, osl])

                    for plane in range(4):
                        q = dq.tile([P, 512], i32, tag="q")
                        nc.vector.tensor_scalar(
                            q[:],
                            t_pk[:],
                            4 * plane,
                            0xF,
                            mybir.AluOpType.logical_shift_right,
                            mybir.AluOpType.bitwise_and,
                        )
                        qf = dq.tile([P, 512], bf16, tag="qf")
                        nc.vector.tensor_copy(qf[:], q[:])
                        t = dq.tile([P, 512], bf16, tag="t")
                        nc.vector.tensor_tensor(
                            t[:], qf[:], t_sc[:], mybir.AluOpType.mult
                        )
                        nc.vector.tensor_tensor(
                            wt_res[:, wt, plane, osl], t[:], t_wb[:], mybir.AluOpType.add
                        )

            # ---- Stage 2: matmul y^T = W @ x^T (+bias at eviction) ----
            kxn_pool = ctx.enter_context(tc.tile_pool(name="kxn", bufs=9))

            kxm_shape = ShapeInfo(pdims=((P, IN // P),), fdims=(OSH,))

            def kxm_producer(nc_, md):
                return wt_res[
                    :, md.k_tile_idx, :, ds(md.m_tile_idx * md.m_tile, md.m_tile)
                ]

            kxn_producer, kxn_shape = dma_from_dram_kxn(kxn_pool, xtp[:])

            def bias_evict(nc_, psum, sbuf, md):
                # On the scalar engine so evictions never queue behind the
                # dequant stream in the DVE FIFO (PSUM-bank head-of-line).
                ob = md.m_tile_idx * 4 + md.m_subtile_idx
                nc_.scalar.activation(
                    sbuf,
                    psum,
                    mybir.ActivationFunctionType.Identity,
                    bias=bias_sb[:, ob : ob + 1],
                    scale=1.0,
                )

            composable_matmul_tile_kernel(
                tc,
                kxm_shape=kxm_shape,
                kxn_shape=kxn_shape,
                output_type=f32,
                kxm_producer=kxm_producer,
                kxn_producer=kxn_producer,
                mxn_consumer=dma_to_dram_mxn(yT[:]),
                mxn_subtile_reducer=bias_evict,
                psum_n_bufs=2,
                temps_n_bufs=2,
            )

    nc.compile()
    return nc


def _get_compiled():
    if "nc" not in _COMPILED:
        _COMPILED["nc"] = _build_nc()
    return _COMPILED["nc"]


def _marshal(input, w_packed, w_scale, w_bias, bias):
    import ml_dtypes

    bf16 = ml_dtypes.bfloat16
    x = np.ascontiguousarray(input, dtype=np.float32).reshape(BS, IN)
    # x^T rows permuted so in' = (wt*4+plane)*128 + p <- original 4*(wt*128+p)+plane
    xt = x.T  # [IN, BS], row index = original in = 4*w + plane, w = wt*128 + p
    xtp = np.ascontiguousarray(
        xt.reshape(NKT, P, 4, BS).transpose(0, 2, 1, 3).reshape(IN, BS).astype(bf16)
    )
    in_maps = []
    for c in range(NCORES):
        osl = slice(c * OSH, (c + 1) * OSH)
        wp = w_packed[osl].reshape(OSH, NW)
        sc = w_scale[osl].reshape(OSH, G).astype(bf16)
        wb = w_bias[osl].reshape(OSH, G).astype(bf16)
        in_maps.append(
            {
                "xtp": xtp,
                "wpkT": np.ascontiguousarray(wp.T),
                "scx": np.ascontiguousarray(np.repeat(sc.T, NW // G, axis=0)),
                "wbx": np.ascontiguousarray(np.repeat(wb.T, NW // G, axis=0)),
                "bias": np.ascontiguousarray(
                    bias[osl].reshape(OSH // P, P).T, dtype=np.float32
                ),
            }
        )
    return in_maps


def kernel(input, w_packed, w_scale, w_bias, bias, _trace=False, _trace_kwargs=None):
    from concourse.bass_utils import run_bass_kernel_spmd

    nc = _get_compiled()
    in_maps = _marshal(input, w_packed, w_scale, w_bias, bias)
    res = run_bass_kernel_spmd(
        nc,
        in_maps,
        core_ids=list(range(NCORES)),
        trace=_trace,
        **(_trace_kwargs or {}),
    )
    out = np.empty((BS, OUT), dtype=np.float32)
    for c in range(NCORES):
        out[:, c * OSH : (c + 1) * OSH] = res.results[c]["yT"].T
    out = out.reshape(B, S, OUT)
    if _trace:
        return out, res
    return out


# revision 13
# speedup vs baseline: 1.2411x; 1.0090x over previous
"""GroupQuantLinear on 8 Trainium2 NeuronCores.

y[b,s,o] = x[b,s,:] @ W[o,:] + bias[o], where W is dequantized on-device from
4-bit packed weights with per-(o, group) affine scale/bias (groups of 256 along
the 4096-wide input dim).

Sharding: tensor-parallel on out_features (8 shards of 2048 rows); x replicated.

Per-core kernel (Bass/Tile), transpose-free, W-stationary design:
  The packed words are transposed on the HOST (layout-only) to [NW, OSH], so
  the on-chip nibble unpack lands directly in [in', out] orientation -- no PE
  transposes.  Per-(o, group) scale/bias are host-expanded to per-word rows
  (pure broadcast) so dequant is three 16-bit-rate DVE ops per tile: unpack
  (fused shift+and, bf16 out), t = q * scale, w = t + wbias, written to a
  DRAM W^T tensor laid out so matmul-side DMAs are contiguous.

  Matmul computes y^T = W @ x^T: kxm = W^T (cached per o-quarter m-tile),
  kxn = x^T bf16 streamed (host-cast, row order in' = ksub*128 + p matching
  the unpack order).  m-tiles align 1:1 with dequant o-quarters, so the first
  quarter unlocks a full 218 us row of PE work and the rest of dequant hides
  under it.  PSUM partitions carry o, so the output bias is a per-partition
  tensor_scalar add at eviction.  Host un-transposes y^T.

in' ordering: global k-subtile ksub = wt*4 + plane (wt = 128-word tile of the
packed words, plane = nibble index), so in' = wt*512 + plane*128 + p maps to
original input index 4*(wt*128 + p) + plane.
"""

import numpy as np

B, S, IN, OUT, G = 2, 2048, 4096, 16384, 16
NCORES = 8
OSH = OUT // NCORES       # 2048 out rows per core
BS = B * S                # 4096
NW = IN // 4              # 1024 packed int32 words per out row
P = 128
NKT = IN // 512           # 8 K tiles of 512 (4 k-subtiles each)
NQ = OSH // 512           # 4 o-quarters = matmul m-tiles

_COMPILED = {}


def _build_nc():
    from contextlib import ExitStack

    import concourse.bass as bass
    import concourse.mybir as mybir
    import concourse.tile as tile
    from concourse import bacc
    from concourse.bass import ds, ts
    from concourse.kernels.tile_matmul import (
        ShapeInfo,
        composable_matmul_tile_kernel,
        dma_from_dram_kxn,
        dma_to_dram_mxn,
    )

    f32 = mybir.dt.float32
    bf16 = mybir.dt.bfloat16
    u16 = mybir.dt.uint16

    nc = bacc.Bacc(None, target_bir_lowering=False)

    xtp = nc.dram_tensor("xtp", [IN, BS], bf16, kind="ExternalInput")
    wpkT = nc.dram_tensor("wpkT", [NW, OSH], u16, kind="ExternalInput")
    scx = nc.dram_tensor("scx", [NW, OSH], bf16, kind="ExternalInput")
    wbx = nc.dram_tensor("wbx", [NW, OSH], bf16, kind="ExternalInput")
    bias = nc.dram_tensor("bias", [P, OSH // P], f32, kind="ExternalInput")
    yT = nc.dram_tensor("yT", [OSH, BS], f32, kind="ExternalOutput")

    with tile.TileContext(nc) as tc:
        with ExitStack() as ctx:
            const = ctx.enter_context(tc.tile_pool(name="const", bufs=1))
            dq = ctx.enter_context(tc.tile_pool(name="dq", bufs=3))

            # output bias, per-partition: bias_sb[p, j] = bias[j*128 + p]
            bias_sb = const.tile([P, OSH // P], f32)
            nc.sync.dma_start(bias_sb[:], bias[:])

            # W^T fully resident in SBUF (128 KB/partition): [p, kt, ksub, o]
            wt_res = const.tile([P, NKT, 4, OSH], bf16)

            # ---- Stage 1: dequant (o-quarter-major = matmul m-tile order) ----
            for j in range(NQ):
                osl = ts(j, 512)
                for wt in range(NKT):
                    rsl = ts(wt, P)
                    t_pk = dq.tile([P, 512], u16, tag="pk")
                    nc.sync.dma_start(t_pk[:], wpkT[rsl, osl])
                    t_sc = dq.tile([P, 512], bf16, tag="sc")
                    nc.sync.dma_start(t_sc[:], scx[rsl, osl])
                    t_wb = dq.tile([P, 512], bf16, tag="wb")
                    nc.sync.dma_start(t_wb[:], wbx[rsl, osl])

                    for plane in range(4):
                        q = dq.tile([P, 512], u16, tag="q")
                        nc.vector.tensor_scalar(
                            q[:],
                            t_pk[:],
                            4 * plane,
                            0xF,
                            mybir.AluOpType.logical_shift_right,
                            mybir.AluOpType.bitwise_and,
                        )
                        qf = dq.tile([P, 512], bf16, tag="qf")
                        nc.vector.tensor_copy(qf[:], q[:])
                        t = dq.tile([P, 512], bf16, tag="t")
                        nc.vector.tensor_tensor(
                            t[:], qf[:], t_sc[:], mybir.AluOpType.mult
                        )
                        nc.vector.tensor_tensor(
                            wt_res[:, wt, plane, osl], t[:], t_wb[:], mybir.AluOpType.add
                        )

            # ---- Stage 2: matmul y^T = W @ x^T (+bias at eviction) ----
            kxn_pool = ctx.enter_context(tc.tile_pool(name="kxn", bufs=9))

            kxm_shape = ShapeInfo(pdims=((P, IN // P),), fdims=(OSH,))

            def kxm_producer(nc_, md):
                return wt_res[
                    :, md.k_tile_idx, :, ds(md.m_tile_idx * md.m_tile, md.m_tile)
                ]

            kxn_producer, kxn_shape = dma_from_dram_kxn(kxn_pool, xtp[:])

            def bias_evict(nc_, psum, sbuf, md):
                # On the scalar engine so evictions never queue behind the
                # dequant stream in the DVE FIFO (PSUM-bank head-of-line).
                ob = md.m_tile_idx * 4 + md.m_subtile_idx
                nc_.scalar.activation(
                    sbuf,
                    psum,
                    mybir.ActivationFunctionType.Identity,
                    bias=bias_sb[:, ob : ob + 1],
                    scale=1.0,
                )

            composable_matmul_tile_kernel(
                tc,
                kxm_shape=kxm_shape,
                kxn_shape=kxn_shape,
                output_type=f32,
                kxm_producer=kxm_producer,
                kxn_producer=kxn_producer,
                mxn_consumer=dma_to_dram_mxn(yT[:]),
                mxn_subtile_reducer=bias_evict,
                psum_n_bufs=2,
                temps_n_bufs=2,
            )

    nc.compile()
    return nc


def _get_compiled():
    if "nc" not in _COMPILED:
        _COMPILED["nc"] = _build_nc()
    return _COMPILED["nc"]


def _marshal(input, w_packed, w_scale, w_bias, bias):
    import ml_dtypes

    bf16 = ml_dtypes.bfloat16
    x = np.ascontiguousarray(input, dtype=np.float32).reshape(BS, IN)
    # x^T rows permuted so in' = (wt*4+plane)*128 + p <- original 4*(wt*128+p)+plane
    xt = x.T  # [IN, BS], row index = original in = 4*w + plane, w = wt*128 + p
    xtp = np.ascontiguousarray(
        xt.reshape(NKT, P, 4, BS).transpose(0, 2, 1, 3).reshape(IN, BS).astype(bf16)
    )
    in_maps = []
    for c in range(NCORES):
        osl = slice(c * OSH, (c + 1) * OSH)
        wp = w_packed[osl].reshape(OSH, NW)
        sc = w_scale[osl].reshape(OSH, G).astype(bf16)
        wb = w_bias[osl].reshape(OSH, G).astype(bf16)
        in_maps.append(
            {
                "xtp": xtp,
                "wpkT": np.ascontiguousarray(wp.T.astype(np.uint16)),
                "scx": np.ascontiguousarray(np.repeat(sc.T, NW // G, axis=0)),
                "wbx": np.ascontiguousarray(np.repeat(wb.T, NW // G, axis=0)),
                "bias": np.ascontiguousarray(
                    bias[osl].reshape(OSH // P, P).T, dtype=np.float32
                ),
            }
        )
    return in_maps


def kernel(input, w_packed, w_scale, w_bias, bias, _trace=False, _trace_kwargs=None):
    from concourse.bass_utils import run_bass_kernel_spmd

    nc = _get_compiled()
    in_maps = _marshal(input, w_packed, w_scale, w_bias, bias)
    res = run_bass_kernel_spmd(
        nc,
        in_maps,
        core_ids=list(range(NCORES)),
        trace=_trace,
        **(_trace_kwargs or {}),
    )
    out = np.empty((BS, OUT), dtype=np.float32)
    for c in range(NCORES):
        out[:, c * OSH : (c + 1) * OSH] = res.results[c]["yT"].T
    out = out.reshape(B, S, OUT)
    if _trace:
        return out, res
    return out
